# revision 2
# baseline (speedup 1.0000x reference)
"""TRN2 Bass kernel for CNF (continuous normalizing flow) forward pass.

Math: RK4 over t_span (16 steps x 4 stages) of the augmented dynamics
  dx/dt = f(t,x) = W3^T tanh(W2^T tanh(W1x^T x + t w1t + b1) + b2) + b3
  dlogdet/dt = -trace(df/dx)
with the exact trace computed analytically:
  trace_b = sum_{k,m} g1[b,k] M[k,m] g2[b,m],  M = W2 ⊙ (W1x^T @ W3^T)
  g1 = 1-h1^2, g2 = 1-h2^2
so each evaluation is 4 matmuls' worth of work (L1, L2, L3, trace) instead
of D forward-mode JVPs.

Layout: activations kept transposed (feature, batch) so every matmul is
lhsT(=weights).T @ rhs(=activations) with contraction on the partition dim.
Data parallel over 8 cores, B_local = 512 = one free-dim tile.

All scalar/bias/time-feature terms (b1 + t*w1t, b2, every fold of b3
through the RK stage shifts) are precomputed on host into small per-stage
bias tables applied for free via the ACT bias port; the b3 drift of the
stored state is corrected once at the end (zfix).
"""

import numpy as np
from contextlib import ExitStack

import concourse.bass as bass
import concourse.mybir as mybir
from concourse import bacc
from concourse.tile import TileContext
from concourse.bass_utils import run_bass_kernel_spmd

B, D, H = 4096, 64, 256
NCORES = 8
BL = B // NCORES          # 512 samples per core
NSTEPS = 16               # len(t_span) - 1
NEVALS = 4 * NSTEPS

F32 = mybir.dt.float32
F32R = mybir.dt.float32r
AF = mybir.ActivationFunctionType
ALU = mybir.AluOpType

# engine assignment for elementwise stages (tune via profile)
ENG_SQ1 = "scalar"    # S1 = H1^2
ENG_RSUB1 = "gpsimd"  # G1 = 1 - S1
ENG_SQ2 = "vector"    # S2 = H2^2
ENG_RSUB2 = "gpsimd"  # G2 = 1 - S2
ENG_Q = "vector"      # Q = P * G2

_NC_CACHE = {}


def _ts(nc, eng, out, in0, s1, s2, op0, op1):
    getattr(nc, eng).tensor_scalar(out, in0, s1, s2, op0, op1)


def _tt(nc, eng, out, in0, in1, op):
    getattr(nc, eng).tensor_tensor(out, in0, in1, op)


def _build():
    nc = bacc.Bacc(target_bir_lowering=False)

    xT = nc.declare_dram_parameter("xT", [D, BL], F32R, isOutput=False)
    w1 = nc.declare_dram_parameter("w1", [D, H], F32R, isOutput=False)
    w2 = nc.declare_dram_parameter("w2", [2, 128, H], F32R, isOutput=False)
    w3 = nc.declare_dram_parameter("w3", [2, 128, D], F32R, isOutput=False)
    mmat = nc.declare_dram_parameter("mmat", [2, 128, H], F32R, isOutput=False)
    b1t = nc.declare_dram_parameter("b1t", [2, 128, NEVALS], F32, isOutput=False)
    b2t = nc.declare_dram_parameter("b2t", [2, 128, 1], F32, isOutput=False)
    onw = nc.declare_dram_parameter("onw", [128, 2 * NSTEPS], F32R, isOutput=False)
    zfx = nc.declare_dram_parameter("zfx", [D, 1], F32, isOutput=False)
    cvec = nc.declare_dram_parameter("cvec", [1, NEVALS], F32, isOutput=False)
    out = nc.declare_dram_parameter("out", [D + 1, BL], F32, isOutput=True)

    with TileContext(nc) as tc, ExitStack() as ctx:
        cp = ctx.enter_context(tc.tile_pool(name="cp", bufs=1))
        sb = ctx.enter_context(tc.tile_pool(name="sb", bufs=2))
        st = ctx.enter_context(tc.tile_pool(name="st", bufs=2))
        psA1 = ctx.enter_context(tc.tile_pool(name="psA1", bufs=1, space="PSUM"))
        psA2 = ctx.enter_context(tc.tile_pool(name="psA2", bufs=1, space="PSUM"))
        psO = ctx.enter_context(tc.tile_pool(name="psO", bufs=1, space="PSUM"))
        psP = ctx.enter_context(tc.tile_pool(name="psP", bufs=1, space="PSUM"))
        psT = ctx.enter_context(tc.tile_pool(name="psT", bufs=1, space="PSUM"))

        w1c = cp.tile_from(w1.ap())
        w2c = [cp.tile_from(w2.ap()[k], name=f"w2c{k}") for k in range(2)]
        w3c = [cp.tile_from(w3.ap()[k], name=f"w3c{k}") for k in range(2)]
        mmc = [cp.tile_from(mmat.ap()[k], name=f"mmc{k}") for k in range(2)]
        b1c = [cp.tile_from(b1t.ap()[c], name=f"b1c{c}") for c in range(2)]
        b2c = [cp.tile_from(b2t.ap()[c], name=f"b2c{c}") for c in range(2)]
        onwc = cp.tile_from(onw.ap())
        zfxc = cp.tile_from(zfx.ap())
        cvecc = cp.tile_from(cvec.ap())
        x0 = cp.tile_from(xT.ap())

        TR = psT.tile([1, BL], F32)

        y = x0
        ystage = None
        acc = None

        for s in range(NSTEPS):
            for j in range(4):
                e = 4 * s + j
                rhs = y if j == 0 else ystage

                # L1: A1^T = W1x^T @ y_stage^T   (2x K=64,M=128,N=512)
                A1 = psA1.tile([128, 2 * BL], F32, name="A1")
                for m in range(2):
                    nc.tensor.matmul(
                        A1[:, m * BL:(m + 1) * BL],
                        lhsT=w1c[:, m * 128:(m + 1) * 128],
                        rhs=rhs[:, :], start=True, stop=True)

                # H1 = tanh(A1 + bias1[e])   (per-chunk for the bias column)
                H1 = sb.tile([128, 2 * BL], F32R, name="H1")
                for c in range(2):
                    nc.scalar.activation(
                        H1[:, c * BL:(c + 1) * BL], A1[:, c * BL:(c + 1) * BL],
                        AF.Tanh, bias=b1c[c][:, e:e + 1])

                # G1 = 1 - H1^2
                S1 = sb.tile([128, 2 * BL], F32R, name="S1")
                if ENG_SQ1 == "scalar":
                    nc.scalar.activation(S1[:, :], H1[:, :], AF.Square)
                else:
                    _tt(nc, ENG_SQ1, S1[:, :], H1[:, :], H1[:, :], ALU.mult)
                G1 = sb.tile([128, 2 * BL], F32R, name="G1")
                _ts(nc, ENG_RSUB1, G1[:, :], S1[:, :], -1.0, 1.0, ALU.mult, ALU.add)

                # L2: A2^T = W2^T @ H1^T  (4x K=128,M=128,N=512)
                A2 = psA2.tile([128, 2 * BL], F32, name="A2")
                for m in range(2):
                    for k in range(2):
                        nc.tensor.matmul(
                            A2[:, m * BL:(m + 1) * BL],
                            lhsT=w2c[k][:, m * 128:(m + 1) * 128],
                            rhs=H1[:, k * BL:(k + 1) * BL],
                            start=(k == 0), stop=(k == 1))

                # H2 = tanh(A2 + b2)
                H2 = sb.tile([128, 2 * BL], F32R, name="H2")
                for c in range(2):
                    nc.scalar.activation(
                        H2[:, c * BL:(c + 1) * BL], A2[:, c * BL:(c + 1) * BL],
                        AF.Tanh, bias=b2c[c][:, 0:1])

                # G2 = 1 - H2^2
                S2 = sb.tile([128, 2 * BL], F32R, name="S2")
                if ENG_SQ2 == "scalar":
                    nc.scalar.activation(S2[:, :], H2[:, :], AF.Square)
                else:
                    _tt(nc, ENG_SQ2, S2[:, :], H2[:, :], H2[:, :], ALU.mult)
                G2 = sb.tile([128, 2 * BL], F32R, name="G2")
                _ts(nc, ENG_RSUB2, G2[:, :], S2[:, :], -1.0, 1.0, ALU.mult, ALU.add)

                # L3: OUT^T = W3^T @ H2^T  (2x K=128,M=64 accumulated)
                OUTp = psO.tile([64, BL], F32, name="OUTp")
                for k in range(2):
                    nc.tensor.matmul(
                        OUTp[:, :], lhsT=w3c[k][:, :],
                        rhs=H2[:, k * BL:(k + 1) * BL],
                        start=(k == 0), stop=(k == 1))

                # P^T = M^T @ G1^T  (4x K=128,M=128,N=512)
                Pp = psP.tile([128, 2 * BL], F32, name="Pp")
                for m in range(2):
                    for k in range(2):
                        nc.tensor.matmul(
                            Pp[:, m * BL:(m + 1) * BL],
                            lhsT=mmc[k][:, m * 128:(m + 1) * 128],
                            rhs=G1[:, k * BL:(k + 1) * BL],
                            start=(k == 0), stop=(k == 1))

                # Q = P ⊙ G2 ; logdet accumulates  -w_e * colsum(Q)  in PSUM
                Q = sb.tile([128, 2 * BL], F32R, name="Q")
                _tt(nc, ENG_Q, Q[:, :], Pp[:, :].bitcast(F32R), G2[:, :], ALU.mult)
                wcol = 2 * s + (0 if j in (0, 3) else 1)
                for k in range(2):
                    nc.tensor.matmul(
                        TR[:, :], lhsT=onwc[:, wcol:wcol + 1],
                        rhs=Q[:, k * BL:(k + 1) * BL],
                        start=(e == 0 and k == 0),
                        stop=(e == NEVALS - 1 and k == 1),
                        skip_group_check=True)

                # RK4 state updates (single fused axpy ops on DVE)
                # cvec columns: [c_next(h/2,h/2,h,0), w(h/6,h/3,h/3,h/6)] --
                # passed as immediates, baked per step on host via closure
                cj = _RK_CJ[e]
                wj = _RK_WJ[e]
                opsum = OUTp[:, :].bitcast(F32R)
                if j < 3:
                    ystage = st.tile([D, BL], F32R, name="ystage", tag="ystage")
                    nc.vector.scalar_tensor_tensor(
                        ystage[:, :], opsum, cj, y[:, :], ALU.mult, ALU.add)
                if j == 0:
                    acc = st.tile([D, BL], F32R, name="acc", tag="acc")
                    nc.vector.scalar_tensor_tensor(
                        acc[:, :], opsum, wj, y[:, :], ALU.mult, ALU.add)
                elif j < 3:
                    acc2 = st.tile([D, BL], F32R, name="acc2", tag="acc2")
                    nc.vector.scalar_tensor_tensor(
                        acc2[:, :], opsum, wj, acc[:, :], ALU.mult, ALU.add)
                    acc = acc2
                else:
                    ynew = st.tile([D, BL], F32R, name="ynew", tag="ynew")
                    nc.vector.scalar_tensor_tensor(
                        ynew[:, :], opsum, wj, acc[:, :], ALU.mult, ALU.add)
                    y = ynew

        # z = y + zfix (accumulated b3 drift), logdet from PSUM
        zout = st.tile([D, BL], F32, name="zout", tag="zout")
        nc.vector.tensor_scalar(zout[:, :], y[:, :].bitcast(F32), zfxc[:, 0:1],
                                None, ALU.add)
        nc.sync.dma_start(out=out.ap()[0:D, :], in_=zout[:, :])
        ld = st.tile([1, BL], F32, name="ld", tag="ld")
        nc.vector.tensor_copy(ld[:, :], TR[:, :])
        nc.sync.dma_start(out=out.ap()[D:D + 1, :], in_=ld[:, :])

    nc.finalize()
    return nc


_RK_CJ = [0.0] * NEVALS  # stage-advance coeff for stages 1..3 of each step
_RK_WJ = [0.0] * NEVALS  # RK accumulation weight


def _set_rk_tables(t_span):
    hs = np.diff(np.asarray(t_span, np.float64))
    for s in range(NSTEPS):
        h = float(hs[s])
        for j in range(4):
            e = 4 * s + j
            _RK_CJ[e] = [h / 2, h / 2, h, 0.0][j]
            _RK_WJ[e] = [h / 6, h / 3, h / 3, h / 6][j]


def _host_prep(x, t_span, W1, b1, W2, b2, W3, b3):
    f64 = np.float64
    t_span = np.asarray(t_span, f64)
    W1 = np.asarray(W1, f64)
    W2_ = np.asarray(W2, f64)
    W3_ = np.asarray(W3, f64)
    b1 = np.asarray(b1, f64)
    b2 = np.asarray(b2, f64)
    b3 = np.asarray(b3, f64)
    W1x = W1[:D]          # (D, H)
    w1t = W1[D]           # (H,)
    M = W2_ * (W1x.T @ W3_.T)

    hs = np.diff(t_span)
    bias1 = np.zeros((H, NEVALS), f64)
    onw = np.zeros((128, 2 * NSTEPS), f64)
    F = np.zeros(D, f64)
    for s in range(NSTEPS):
        h = float(hs[s])
        t0 = float(t_span[s])
        stage_t = [t0, t0 + h / 2, t0 + h / 2, t0 + h]
        stage_c = [0.0, h / 2, h / 2, h]
        for j in range(4):
            e = 4 * s + j
            shift = F + stage_c[j] * b3
            bias1[:, e] = b1 + stage_t[j] * w1t + W1x.T @ shift
        onw[:, 2 * s] = -h / 6
        onw[:, 2 * s + 1] = -h / 3
        F = F + h * b3

    f32 = np.float32
    prep = {
        "w1": np.ascontiguousarray(W1x, f32),
        "w2": np.ascontiguousarray(
            np.asarray(W2_, f32).reshape(2, 128, H)),
        "w3": np.ascontiguousarray(
            np.asarray(W3_, f32).reshape(2, 128, D)),
        "mmat": np.ascontiguousarray(np.asarray(M, f32).reshape(2, 128, H)),
        "b1t": np.ascontiguousarray(
            np.asarray(bias1, f32).reshape(2, 128, NEVALS)),
        "b2t": np.ascontiguousarray(
            np.asarray(b2, f32).reshape(2, 128, 1)),
        "onw": np.ascontiguousarray(onw, f32),
        "zfx": np.ascontiguousarray(F.reshape(D, 1), f32),
        "cvec": np.zeros((1, NEVALS), f32),
    }
    return prep


def run(inputs, trace=False):
    x = np.asarray(inputs["x"], np.float32)
    t_span = np.asarray(inputs["t_span"], np.float32)
    assert x.shape == (B, D) and t_span.shape == (NSTEPS + 1,)

    _set_rk_tables(t_span)
    key = t_span.tobytes()
    if key not in _NC_CACHE:
        _NC_CACHE[key] = _build()
    nc = _NC_CACHE[key]

    prep = _host_prep(x, t_span, inputs["W1"], inputs["b1"], inputs["W2"],
                      inputs["b2"], inputs["W3"], inputs["b3"])

    in_maps = []
    for i in range(NCORES):
        shard = np.ascontiguousarray(x[i * BL:(i + 1) * BL].T)  # (D, BL)
        in_maps.append({"xT": shard, **prep})

    res = run_bass_kernel_spmd(nc, in_maps, core_ids=list(range(NCORES)),
                               trace=trace)
    z = np.empty((B, D), np.float32)
    ld = np.empty((B,), np.float32)
    for i in range(NCORES):
        o = res.results[i]["out"]
        z[i * BL:(i + 1) * BL] = o[:D].T
        ld[i * BL:(i + 1) * BL] = o[D]
    return z, ld, res


def kernel(**inputs):
    z, ld, _ = run(inputs, trace=False)
    return z, ld
